# revision 2
# baseline (speedup 1.0000x reference)
"""LocalWindowAttention Trainium2 kernel (Bass/Tile), 8-core SPMD — v2.

Problem: x[B=4, S=4096, E=512] -> out[B, S, E]
  qkv = x @ W_qkv + b_qkv ; q,k,v = split(qkv)
  scores = (q @ k.T) / sqrt(E), banded mask |i-j| <= 64, softmax
  out = (attn @ v) @ W_out + b_out

Sharding: 8 cores = (batch b in 0..3) x (seq half h in 0..1). Each core owns
2048 query rows and loads a 64-row halo of x on each side (zero-padded at
sequence boundaries), computing q/k/v locally — no collectives.

v2 design (vs the fp32r v1):
  - All matmul operands are bf16 (PSUM accumulation stays fp32). bf16
    streams 1 col/cycle at ANY moving size (fp32r needs >= 256) and gets
    fast weight loads, halving LDWEIGHTS cost.
  - Scores are computed directly TRANSPOSED, per 128-key chunk j:
    scoresT[k in chunk j, q in its 256-query span] = kT_chunk.T @ qT.
    This kills all 32 PE transposes + their PSUM round trips of v1.
  - Softmax without max-subtraction (scores are O(1)); additive band mask
    on DVE in-place in PSUM, then ACT exp -> bf16 exp tile.
  - Row sums via PE: ones-column matmuls with the exp tiles as stationary
    give rowsum[q, 1] directly in natural orientation; DVE reciprocal.
  - Attention is left UNNORMALIZED through attendedT and the output
    projection; the 1/rowsum scale and the folded output bias
    bo' = b_v @ W_out + b_out (valid because softmax rows sum to 1)
    are applied in ONE fused DVE scalar_tensor_tensor on the final
    PSUM->SBUF copy: out = (po * rd) + bo'.
  - q is only projected for the 2048 owned rows (not the halo).
  - attendedT accumulates the 4 e-chunks into a single PSUM bank using
    per-column-range start=True groups (start clears only the has_written
    bits bank-wide; prior ranges' values are complete and unaffected).
  - PE warm-up: a chain of small matmuls on a zeroed SBUF tile issued
    before any DMA-dependent work, so the HAM clock gate reaches 8/8
    before the real matmuls start, and a dummy Exp to preload the ACT
    table set during the DMA head.
"""

import sys

sys.path.insert(0, "/opt/trn_rl_repo")

import ml_dtypes
import numpy as np

import concourse.bass as bass  # noqa: F401  (registers types)
import concourse.tile as tile
from concourse import bacc, mybir
from concourse.bass_utils import run_bass_kernel_spmd

F32 = mybir.dt.float32
BF16 = mybir.dt.bfloat16
NPBF = ml_dtypes.bfloat16

B, S, E = 4, 4096, 512
WINDOW = 64
HALF = S // 2              # 2048 query rows per core
ROWS = HALF + 2 * WINDOW   # 2176 local rows incl. halo
EC = E // 128              # 4 contraction chunks
NT = HALF // 128           # 16 query subtiles per core
NCH = NT + 1               # 17 key chunks per core
WARMUP_MMS = 56

# exp_all column layout: chunk 0 -> 128 cols, chunks 1..15 -> 256, chunk 16 -> 128
OFF = [0] + [128 + 256 * (j - 1) for j in range(1, NCH)]
WID = [128] + [256] * 15 + [128]
EXPW = OFF[16] + 128       # 4096

_NC_CACHE = {}


def _qspan(j):
    """(first qT column, width) of key chunk j's query span."""
    if j == 0:
        return 0, 128
    if j == NCH - 1:
        return HALF - 128, 128
    return 128 * j - 128, 256


def _build():
    nc = bacc.Bacc("TRN2", target_bir_lowering=False, debug=False, num_devices=8)

    xT_d = nc.dram_tensor("xT", [E, ROWS], BF16, kind="ExternalInput")
    wqkv_d = nc.dram_tensor("wqkv", [E, 3 * E], BF16, kind="ExternalInput")
    bq_d = nc.dram_tensor("bq", [128, 8], F32, kind="ExternalInput")
    wo_d = nc.dram_tensor("wout", [E, E], BF16, kind="ExternalInput")
    bob_d = nc.dram_tensor("bob", [128, E], F32, kind="ExternalInput")
    ones_d = nc.dram_tensor("ones", [128, 1], BF16, kind="ExternalInput")
    mask_d = nc.dram_tensor("masks", [128, 512], BF16, kind="ExternalInput")
    out_d = nc.dram_tensor("out", [HALF, E], F32, kind="ExternalOutput")

    ACT = mybir.ActivationFunctionType
    ALU = mybir.AluOpType

    with tile.TileContext(nc) as tc:
        with (
            tc.tile_pool(name="const", bufs=1) as const,
            tc.tile_pool(name="big", bufs=1) as big,
            tc.tile_pool(name="attn", bufs=2) as attn,
            tc.tile_pool(name="ps", bufs=1, space="PSUM") as psp,
        ):
            # ---- SBUF tiles ----
            wq_sb = [const.tile([128, 3 * E], BF16, name=f"wq{e}", tag=f"wq{e}")
                     for e in range(EC)]
            wo_sb = [const.tile([128, E], BF16, name=f"wo{e}", tag=f"wo{e}")
                     for e in range(EC)]
            bq_sb = const.tile([128, 8], F32, name="bq", tag="bq")
            bob_sb = const.tile([128, E], F32, name="bob", tag="bob")
            ones_sb = const.tile([128, 1], BF16, name="ones1", tag="ones1")
            mask_sb = const.tile([128, 512], BF16, name="msk", tag="msk")
            warm_sb = const.tile([128, 64], BF16, name="warm", tag="warm")
            dummy_sb = const.tile([128, 1], F32, name="dmy", tag="dmy")
            rd_all = const.tile([128, NT], F32, name="rd", tag="rd")

            xT = [big.tile([128, ROWS], BF16, name=f"xT{e}", tag=f"xT{e}")
                  for e in range(EC)]
            qT = [big.tile([128, HALF], BF16, name=f"qT{f}", tag=f"qT{f}")
                  for f in range(EC)]
            kT = [big.tile([128, ROWS], BF16, name=f"kT{f}", tag=f"kT{f}")
                  for f in range(EC)]
            v_sb = [big.tile([128, E], BF16, name=f"v{r}", tag=f"v{r}")
                    for r in range(NCH)]
            exp_all = big.tile([128, EXPW], BF16, name="expa", tag="expa")

            # ---- PE warm-up + ACT table preload (no DMA deps) ----
            nc.vector.memset(warm_sb[:], 0.0)
            nc.scalar.activation(out=dummy_sb[:], in_=warm_sb[:, 0:1],
                                 func=ACT.Exp)
            pw = psp.tile([128, 64], F32, name="pwarm", tag="pp")
            for i in range(WARMUP_MMS):
                nc.tensor.matmul(pw[0:64, :], warm_sb[:, 0:64],
                                 warm_sb[:, 0:64], start=True, stop=True)

            # ---- input DMA (sync queue; order = arrival order) ----
            nc.sync.dma_start(out=bq_sb, in_=bq_d[:, :])
            for e in range(EC):
                nc.sync.dma_start(out=wq_sb[e][:, 0:E],
                                  in_=wqkv_d[128 * e:128 * (e + 1), 0:E])
            nc.sync.dma_start(out=mask_sb, in_=mask_d[:, :])
            nc.sync.dma_start(out=ones_sb, in_=ones_d[:, :])
            XS = [(0, 512), (512, 512), (1024, 512), (1536, 512), (2048, 128)]
            for si, (c0, w) in enumerate(XS):
                for e in range(EC):
                    nc.sync.dma_start(out=xT[e][:, c0:c0 + w],
                                      in_=xT_d[128 * e:128 * (e + 1), c0:c0 + w])
                if si == 0:
                    for e in range(EC):
                        nc.sync.dma_start(out=wq_sb[e][:, E:2 * E],
                                          in_=wqkv_d[128 * e:128 * (e + 1), E:2 * E])
                if si == 1:
                    for e in range(EC):
                        nc.sync.dma_start(out=wq_sb[e][:, 2 * E:3 * E],
                                          in_=wqkv_d[128 * e:128 * (e + 1), 2 * E:3 * E])
                if si == 2:
                    for e in range(EC):
                        nc.sync.dma_start(out=wo_sb[e],
                                          in_=wo_d[128 * e:128 * (e + 1), :])
                    nc.sync.dma_start(out=bob_sb, in_=bob_d[:, :])

            # ---- q projection: qT[f][:, c] = q[row 64+c, 128f+p] ----
            for s in range(4):
                for f in range(EC):
                    ps = psp.tile([128, 512], F32, name=f"pq{s}_{f}", tag="pp")
                    for e in range(EC):
                        nc.tensor.matmul(
                            ps[:],
                            wq_sb[e][:, 128 * f:128 * (f + 1)],
                            xT[e][:, 64 + 512 * s:64 + 512 * (s + 1)],
                            start=(e == 0), stop=(e == EC - 1),
                        )
                    nc.scalar.activation(
                        out=qT[f][:, 512 * s:512 * (s + 1)], in_=ps[:],
                        func=ACT.Identity, bias=bq_sb[:, f:f + 1],
                    )

            # ---- k projection over all 2176 rows ----
            for (c0, w) in XS:
                for f in range(EC):
                    ps = psp.tile([128, 512], F32, name=f"pk{c0}_{f}", tag="pp")
                    for e in range(EC):
                        nc.tensor.matmul(
                            ps[:, :w],
                            wq_sb[e][:, E + 128 * f:E + 128 * (f + 1)],
                            xT[e][:, c0:c0 + w],
                            start=(e == 0), stop=(e == EC - 1),
                        )
                    nc.scalar.activation(
                        out=kT[f][:, c0:c0 + w], in_=ps[:, :w],
                        func=ACT.Identity, bias=bq_sb[:, 4 + f:5 + f],
                    )

            # ---- v projection (natural layout, NO bias — folded into bo') ----
            for r in range(NCH):
                ps = psp.tile([128, 512], F32, name=f"pv{r}", tag="pp")
                for e in range(EC):
                    nc.tensor.matmul(
                        ps[:],
                        xT[e][:, 128 * r:128 * (r + 1)],
                        wq_sb[e][:, 2 * E:3 * E],
                        start=(e == 0), stop=(e == EC - 1),
                    )
                nc.vector.tensor_copy(v_sb[r][:], ps[:])

            # ---- attention ----
            def emit_scores(j):
                c0, w = _qspan(j)
                ps = psp.tile([128, 256], F32, name=f"ps_s{j}", tag="sra",
                              bufs=4)
                for e in range(EC):
                    nc.tensor.matmul(
                        ps[:, :w],
                        kT[e][:, 128 * j:128 * (j + 1)],
                        qT[e][:, c0:c0 + w],
                        start=(e == 0), stop=(e == EC - 1),
                    )
                moff = 0 if j == 0 else (384 if j == NCH - 1 else 128)
                nc.vector.tensor_add(ps[:, :w], ps[:, :w],
                                     mask_sb[:, moff:moff + w])
                nc.scalar.activation(out=exp_all[:, OFF[j]:OFF[j] + w],
                                     in_=ps[:, :w], func=ACT.Exp)

            def emit_subtile(t):
                lo = OFF[t] if t == 0 else OFF[t] + 128
                ro = OFF[t + 1]
                # row sums [q, 1] with exp tiles stationary
                pr = psp.tile([128, 1], F32, name=f"ps_r{t}", tag="sra",
                              bufs=4)
                nc.tensor.matmul(pr[:], exp_all[:, lo:lo + 128], ones_sb[:],
                                 start=True, stop=False)
                nc.tensor.matmul(pr[:], exp_all[:, ro:ro + 128], ones_sb[:],
                                 start=False, stop=True)
                nc.vector.reciprocal(rd_all[:, t:t + 1], pr[:])
                # attendedT: 4 e-chunk col ranges in one PSUM bank
                pa = psp.tile([128, 512], F32, name=f"ps_a{t}", tag="sra",
                              bufs=4)
                for c in range(EC):
                    nc.tensor.matmul(
                        pa[:, 128 * c:128 * (c + 1)],
                        v_sb[t][:, 128 * c:128 * (c + 1)],
                        exp_all[:, lo:lo + 128],
                        start=True, stop=False,
                    )
                    nc.tensor.matmul(
                        pa[:, 128 * c:128 * (c + 1)],
                        v_sb[t + 1][:, 128 * c:128 * (c + 1)],
                        exp_all[:, ro:ro + 128],
                        start=False, stop=True,
                    )
                at = attn.tile([128, 512], BF16, name=f"attT{t}", tag="attT")
                nc.vector.tensor_copy(at[:], pa[:])
                # output projection + fused (x * 1/rowsum) + bo'
                po = psp.tile([128, 512], F32, name=f"ps_o{t}", tag="o",
                              bufs=2)
                for c in range(EC):
                    nc.tensor.matmul(
                        po[:],
                        at[:, 128 * c:128 * (c + 1)],
                        wo_sb[c][:],
                        start=(c == 0), stop=(c == EC - 1),
                    )
                ost = attn.tile([128, 512], F32, name=f"ost{t}", tag="ost")
                nc.vector.scalar_tensor_tensor(
                    ost[:], po[:], rd_all[:, t:t + 1], bob_sb[:],
                    ALU.mult, ALU.add,
                )
                nc.sync.dma_start(out=out_d[128 * t:128 * (t + 1), :],
                                  in_=ost[:])

            for j in range(NCH):
                emit_scores(j)
                if j >= 2:
                    emit_subtile(j - 2)
            emit_subtile(NT - 1)

    nc.compile()
    return nc


def _get_nc():
    if "nc" not in _NC_CACHE:
        _NC_CACHE["nc"] = _build()
    return _NC_CACHE["nc"]


def _prep_shared(W_qkv, b_qkv, W_out, b_out):
    scale = 1.0 / np.sqrt(np.float32(E))
    w = np.array(W_qkv, dtype=np.float32, copy=True)
    w[:, :E] *= scale
    b = np.array(b_qkv, dtype=np.float32, copy=True)
    b[:E] *= scale
    bq_col = np.empty((128, 8), dtype=np.float32)
    for f in range(EC):
        bq_col[:, f] = b[128 * f:128 * (f + 1)]
        bq_col[:, 4 + f] = b[E + 128 * f:E + 128 * (f + 1)]
    b_v = b[2 * E:]
    bo_p = (b_v @ np.asarray(W_out, np.float32)
            + np.asarray(b_out, np.float32)).astype(np.float32)
    return {
        "wqkv": np.ascontiguousarray(w.astype(NPBF)),
        "bq": np.ascontiguousarray(bq_col),
        "wout": np.ascontiguousarray(np.asarray(W_out, np.float32).astype(NPBF)),
        "bob": np.ascontiguousarray(np.tile(bo_p[None, :], (128, 1))),
        "ones": np.ones((128, 1), dtype=NPBF),
    }


def _masks_for(h: int) -> np.ndarray:
    """Additive masks [128, 512] bf16: [chunk0 | interior | chunk16].

    Tile element (a, c) of chunk j is key local-row L = 128j + a against
    query local-row r = span_start(j) + 64 + c... computed from first
    principles below.  Valid iff |global q - global k| <= WINDOW and the
    key's global position is inside [0, S)."""
    NEG = np.float32(-1e30)

    def chunk_mask(j, h):
        c0, w = _qspan(j)
        L = 128 * j + np.arange(128)[:, None]      # local key row
        r = 64 + (c0 + np.arange(w))[None, :]      # local query row
        valid = np.abs(r - L) <= WINDOW
        if h == 0:
            valid = valid & (L >= WINDOW)          # global key >= 0
        else:
            valid = valid & (L < ROWS - WINDOW)    # global key < S
        return np.where(valid, np.float32(0.0), NEG)

    m0 = chunk_mask(0, h)                  # [128, 128]
    mi = chunk_mask(8, h)                  # interior pattern, j-independent
    m16 = chunk_mask(NCH - 1, h)           # [128, 128]
    return np.ascontiguousarray(
        np.concatenate([m0, mi, m16], axis=1).astype(NPBF))


def _install_ntff_shim():
    """The agent image's antenv lacks axon_hooks; synthesize it from the
    boot module's ctypes NTFF driver so trace=True can capture HW timing."""
    import types
    if "antenv.axon_hooks" in sys.modules:
        return
    try:
        from trn_agent_boot.trn_boot import _ntff_profile_via_ctypes
        hook = _ntff_profile_via_ctypes("/opt/axon/libaxon_pjrt.so")
    except Exception:
        hook = None
    mod = types.ModuleType("antenv.axon_hooks")
    mod.get_axon_ntff_profile_hook = lambda: hook
    mod.set_axon_ntff_profile_hook = lambda h: None
    sys.modules["antenv.axon_hooks"] = mod
    # avoid S3 artifact upload attempts during local profile processing
    try:
        from concourse import bass_utils as _bu
        _bu.upload_artifacts = lambda tmpdir: tmpdir
    except Exception:
        pass


def kernel(x, W_qkv, b_qkv, W_out, b_out, _trace=False):
    x = np.asarray(x, dtype=np.float32)
    nc = _get_nc()
    shared = _prep_shared(W_qkv, b_qkv, W_out, b_out)
    masks = [_masks_for(0), _masks_for(1)]

    in_maps = []
    for core in range(8):
        b, h = divmod(core, 2)
        lo = h * HALF - WINDOW
        hi = lo + ROWS
        xh = np.zeros((ROWS, E), dtype=np.float32)
        s0, s1 = max(lo, 0), min(hi, S)
        xh[s0 - lo:s1 - lo] = x[b, s0:s1]
        in_maps.append({
            "xT": np.ascontiguousarray(xh.T.astype(NPBF)),
            "masks": masks[h],
            **shared,
        })

    kwargs = {}
    if _trace:
        _install_ntff_shim()
        kwargs = dict(trace=True, trace_cores=[0])
    res = run_bass_kernel_spmd(nc, in_maps, core_ids=list(range(8)), **kwargs)

    out = np.empty((B, S, E), dtype=np.float32)
    for core in range(8):
        b, h = divmod(core, 2)
        out[b, h * HALF:(h + 1) * HALF] = res.results[core]["out"]
    if _trace:
        return out, res
    return out


# revision 4
# speedup vs baseline: 1.3853x; 1.3853x over previous
"""LocalWindowAttention Trainium2 kernel (Bass/Tile), 8-core SPMD — v2.

Problem: x[B=4, S=4096, E=512] -> out[B, S, E]
  qkv = x @ W_qkv + b_qkv ; q,k,v = split(qkv)
  scores = (q @ k.T) / sqrt(E), banded mask |i-j| <= 64, softmax
  out = (attn @ v) @ W_out + b_out

Sharding: 8 cores = (batch b in 0..3) x (seq half h in 0..1). Each core owns
2048 query rows and loads a 64-row halo of x on each side (zero-padded at
sequence boundaries), computing q/k/v locally — no collectives.

v2 design (vs the fp32r v1):
  - All matmul operands are bf16 (PSUM accumulation stays fp32). bf16
    streams 1 col/cycle at ANY moving size (fp32r needs >= 256) and gets
    fast weight loads, halving LDWEIGHTS cost.
  - Scores are computed directly TRANSPOSED, per 128-key chunk j:
    scoresT[k in chunk j, q in its 256-query span] = kT_chunk.T @ qT.
    This kills all 32 PE transposes + their PSUM round trips of v1.
  - Softmax without max-subtraction (scores are O(1)); additive band mask
    on DVE in-place in PSUM, then ACT exp -> bf16 exp tile.
  - Row sums via PE: ones-column matmuls with the exp tiles as stationary
    give rowsum[q, 1] directly in natural orientation; DVE reciprocal.
  - Attention is left UNNORMALIZED through attendedT and the output
    projection; the 1/rowsum scale and the folded output bias
    bo' = b_v @ W_out + b_out (valid because softmax rows sum to 1)
    are applied in ONE fused DVE scalar_tensor_tensor on the final
    PSUM->SBUF copy: out = (po * rd) + bo'.
  - q is only projected for the 2048 owned rows (not the halo).
  - attendedT accumulates the 4 e-chunks into a single PSUM bank using
    per-column-range start=True groups (start clears only the has_written
    bits bank-wide; prior ranges' values are complete and unaffected).
  - PE warm-up: a chain of small matmuls on a zeroed SBUF tile issued
    before any DMA-dependent work, so the HAM clock gate reaches 8/8
    before the real matmuls start, and a dummy Exp to preload the ACT
    table set during the DMA head.
"""

import sys

sys.path.insert(0, "/opt/trn_rl_repo")

import ml_dtypes
import numpy as np

import concourse.bass as bass  # noqa: F401  (registers types)
import concourse.tile as tile
from concourse import bacc, mybir
from concourse.bass_utils import run_bass_kernel_spmd

F32 = mybir.dt.float32
BF16 = mybir.dt.bfloat16
NPBF = ml_dtypes.bfloat16

B, S, E = 4, 4096, 512
WINDOW = 64
HALF = S // 2              # 2048 query rows per core
ROWS = HALF + 2 * WINDOW   # 2176 local rows incl. halo
EC = E // 128              # 4 contraction chunks
NT = HALF // 128           # 16 query subtiles per core
NCH = NT + 1               # 17 key chunks per core
WARMUP_MMS = 56

# exp_all column layout: chunk 0 -> 128 cols, chunks 1..15 -> 256, chunk 16 -> 128
OFF = [0] + [128 + 256 * (j - 1) for j in range(1, NCH)]
WID = [128] + [256] * 15 + [128]
EXPW = OFF[16] + 128       # 4096

_NC_CACHE = {}


def _qspan(j):
    """(first qT column, width) of key chunk j's query span."""
    if j == 0:
        return 0, 128
    if j == NCH - 1:
        return HALF - 128, 128
    return 128 * j - 128, 256


def _build():
    nc = bacc.Bacc("TRN2", target_bir_lowering=False, debug=False, num_devices=8)

    xT_d = nc.dram_tensor("xT", [E, ROWS], BF16, kind="ExternalInput")
    wqkv_d = nc.dram_tensor("wqkv", [E, 3 * E], BF16, kind="ExternalInput")
    bq_d = nc.dram_tensor("bq", [128, 8], F32, kind="ExternalInput")
    wo_d = nc.dram_tensor("wout", [E, E], BF16, kind="ExternalInput")
    bob_d = nc.dram_tensor("bob", [128, E], F32, kind="ExternalInput")
    ones_d = nc.dram_tensor("ones", [128, 1], BF16, kind="ExternalInput")
    mask_d = nc.dram_tensor("masks", [128, 512], BF16, kind="ExternalInput")
    out_d = nc.dram_tensor("out", [HALF, E], F32, kind="ExternalOutput")

    ACT = mybir.ActivationFunctionType
    ALU = mybir.AluOpType

    with tile.TileContext(nc) as tc:
        with (
            tc.tile_pool(name="const", bufs=1) as const,
            tc.tile_pool(name="big", bufs=1) as big,
            tc.tile_pool(name="attn", bufs=2) as attn,
            tc.tile_pool(name="ps", bufs=1, space="PSUM") as psp,
        ):
            # ---- SBUF tiles ----
            wq_sb = [const.tile([128, 3 * E], BF16, name=f"wq{e}", tag=f"wq{e}")
                     for e in range(EC)]
            wo_sb = [const.tile([128, E], BF16, name=f"wo{e}", tag=f"wo{e}")
                     for e in range(EC)]
            bq_sb = const.tile([128, 8], F32, name="bq", tag="bq")
            bob_sb = const.tile([128, E], F32, name="bob", tag="bob")
            ones_sb = const.tile([128, 1], BF16, name="ones1", tag="ones1")
            mask_sb = const.tile([128, 512], BF16, name="msk", tag="msk")
            warm_sb = const.tile([128, 64], BF16, name="warm", tag="warm")
            dummy_sb = const.tile([128, 1], F32, name="dmy", tag="dmy")
            rd_all = const.tile([128, NT], F32, name="rd", tag="rd")

            xT = [big.tile([128, ROWS], BF16, name=f"xT{e}", tag=f"xT{e}")
                  for e in range(EC)]
            qT = [big.tile([128, HALF], BF16, name=f"qT{f}", tag=f"qT{f}")
                  for f in range(EC)]
            kT = [big.tile([128, ROWS], BF16, name=f"kT{f}", tag=f"kT{f}")
                  for f in range(EC)]
            v_sb = [big.tile([128, E], BF16, name=f"v{r}", tag=f"v{r}")
                    for r in range(NCH)]
            exp_all = big.tile([128, EXPW], BF16, name="expa", tag="expa")

            # ---- PE warm-up + ACT table preload (no DMA deps) ----
            nc.vector.memset(warm_sb[:], 0.0)
            nc.scalar.activation(out=dummy_sb[:], in_=warm_sb[:, 0:1],
                                 func=ACT.Exp)
            pw = psp.tile([128, 64], F32, name="pwarm", tag="o", bufs=2)
            for i in range(WARMUP_MMS):
                nc.tensor.matmul(pw[0:64, :], warm_sb[:, 0:64],
                                 warm_sb[:, 0:64], start=True, stop=True)

            # ---- input DMA across 3 queues ----
            # scalar queue: small consts + q-projection weights (needed first)
            nc.scalar.dma_start(out=bq_sb, in_=bq_d[:, :])
            for e in range(EC):
                nc.scalar.dma_start(out=wq_sb[e][:, 0:E],
                                    in_=wqkv_d[128 * e:128 * (e + 1), 0:E])
            nc.scalar.dma_start(out=mask_sb, in_=mask_d[:, :])
            nc.scalar.dma_start(out=ones_sb, in_=ones_d[:, :])
            # sync queue: xT e-chunks 0-1; gpsimd queue: xT e-chunks 2-3
            XS = [(0, 512), (512, 512), (1024, 512), (1536, 512), (2048, 128)]
            for (c0, w) in XS:
                for e in range(EC):
                    q = nc.sync if e < 2 else nc.gpsimd
                    q.dma_start(out=xT[e][:, c0:c0 + w],
                                in_=xT_d[128 * e:128 * (e + 1), c0:c0 + w])
            # gpsimd queue: remaining weights after xT
            for e in range(EC):
                nc.gpsimd.dma_start(out=wq_sb[e][:, E:2 * E],
                                    in_=wqkv_d[128 * e:128 * (e + 1), E:2 * E])
            for e in range(EC):
                nc.gpsimd.dma_start(out=wq_sb[e][:, 2 * E:3 * E],
                                    in_=wqkv_d[128 * e:128 * (e + 1), 2 * E:3 * E])
            for e in range(EC):
                nc.gpsimd.dma_start(out=wo_sb[e],
                                    in_=wo_d[128 * e:128 * (e + 1), :])
            nc.gpsimd.dma_start(out=bob_sb, in_=bob_d[:, :])

            # ---- q projection: qT[f][:, c] = q[row 64+c, 128f+p] ----
            for s in range(4):
                for f in range(EC):
                    ps = psp.tile([128, 512], F32, name=f"pq{s}_{f}", tag="pp", bufs=3)
                    for e in range(EC):
                        nc.tensor.matmul(
                            ps[:],
                            wq_sb[e][:, 128 * f:128 * (f + 1)],
                            xT[e][:, 64 + 512 * s:64 + 512 * (s + 1)],
                            start=(e == 0), stop=(e == EC - 1),
                        )
                    nc.scalar.activation(
                        out=qT[f][:, 512 * s:512 * (s + 1)], in_=ps[:],
                        func=ACT.Identity, bias=bq_sb[:, f:f + 1],
                    )

            # ---- k projection over all 2176 rows ----
            for (c0, w) in XS:
                for f in range(EC):
                    ps = psp.tile([128, 512], F32, name=f"pk{c0}_{f}", tag="pp", bufs=3)
                    for e in range(EC):
                        nc.tensor.matmul(
                            ps[:, :w],
                            wq_sb[e][:, E + 128 * f:E + 128 * (f + 1)],
                            xT[e][:, c0:c0 + w],
                            start=(e == 0), stop=(e == EC - 1),
                        )
                    nc.scalar.activation(
                        out=kT[f][:, c0:c0 + w], in_=ps[:, :w],
                        func=ACT.Identity, bias=bq_sb[:, 4 + f:5 + f],
                    )

            # ---- v projection (natural layout, NO bias — folded into bo') ----
            for r in range(NCH):
                ps = psp.tile([128, 512], F32, name=f"pv{r}", tag="pp", bufs=3)
                for e in range(EC):
                    nc.tensor.matmul(
                        ps[:],
                        xT[e][:, 128 * r:128 * (r + 1)],
                        wq_sb[e][:, 2 * E:3 * E],
                        start=(e == 0), stop=(e == EC - 1),
                    )
                nc.vector.tensor_copy(v_sb[r][:], ps[:])

            # ---- attention ----
            def emit_scores(j):
                c0, w = _qspan(j)
                ps = psp.tile([128, 256], F32, name=f"ps_s{j}", tag="sra",
                              bufs=3)
                for e in range(EC):
                    nc.tensor.matmul(
                        ps[:, :w],
                        kT[e][:, 128 * j:128 * (j + 1)],
                        qT[e][:, c0:c0 + w],
                        start=(e == 0), stop=(e == EC - 1),
                    )
                moff = 0 if j == 0 else (384 if j == NCH - 1 else 128)
                nc.vector.tensor_add(ps[:, :w], ps[:, :w],
                                     mask_sb[:, moff:moff + w])
                nc.scalar.activation(out=exp_all[:, OFF[j]:OFF[j] + w],
                                     in_=ps[:, :w], func=ACT.Exp)

            def emit_subtile(t):
                lo = OFF[t] if t == 0 else OFF[t] + 128
                ro = OFF[t + 1]
                # row sums [q, 1] with exp tiles stationary
                pr = psp.tile([128, 1], F32, name=f"ps_r{t}", tag="sra",
                              bufs=3)
                nc.tensor.matmul(pr[:], exp_all[:, lo:lo + 128], ones_sb[:],
                                 start=True, stop=False)
                nc.tensor.matmul(pr[:], exp_all[:, ro:ro + 128], ones_sb[:],
                                 start=False, stop=True)
                nc.vector.reciprocal(rd_all[:, t:t + 1], pr[:])
                # attendedT: 4 e-chunk col ranges in one PSUM bank
                pa = psp.tile([128, 512], F32, name=f"ps_a{t}", tag="sra",
                              bufs=3)
                for c in range(EC):
                    nc.tensor.matmul(
                        pa[:, 128 * c:128 * (c + 1)],
                        v_sb[t][:, 128 * c:128 * (c + 1)],
                        exp_all[:, lo:lo + 128],
                        start=True, stop=False,
                    )
                    nc.tensor.matmul(
                        pa[:, 128 * c:128 * (c + 1)],
                        v_sb[t + 1][:, 128 * c:128 * (c + 1)],
                        exp_all[:, ro:ro + 128],
                        start=False, stop=True,
                    )
                at = attn.tile([128, 512], BF16, name=f"attT{t}", tag="attT")
                nc.vector.tensor_copy(at[:], pa[:])
                # output projection + fused (x * 1/rowsum) + bo'
                po = psp.tile([128, 512], F32, name=f"ps_o{t}", tag="o",
                              bufs=2)
                for c in range(EC):
                    nc.tensor.matmul(
                        po[:],
                        at[:, 128 * c:128 * (c + 1)],
                        wo_sb[c][:],
                        start=(c == 0), stop=(c == EC - 1),
                    )
                ost = attn.tile([128, 512], F32, name=f"ost{t}", tag="ost")
                nc.vector.scalar_tensor_tensor(
                    ost[:], po[:], rd_all[:, t:t + 1], bob_sb[:],
                    ALU.mult, ALU.add,
                )
                nc.sync.dma_start(out=out_d[128 * t:128 * (t + 1), :],
                                  in_=ost[:])

            for j in range(NCH):
                emit_scores(j)
                if j >= 2:
                    emit_subtile(j - 2)
            emit_subtile(NT - 1)

    nc.compile()
    return nc


def _get_nc():
    if "nc" not in _NC_CACHE:
        _NC_CACHE["nc"] = _build()
    return _NC_CACHE["nc"]


def _prep_shared(W_qkv, b_qkv, W_out, b_out):
    scale = 1.0 / np.sqrt(np.float32(E))
    w = np.array(W_qkv, dtype=np.float32, copy=True)
    w[:, :E] *= scale
    b = np.array(b_qkv, dtype=np.float32, copy=True)
    b[:E] *= scale
    bq_col = np.empty((128, 8), dtype=np.float32)
    for f in range(EC):
        bq_col[:, f] = b[128 * f:128 * (f + 1)]
        bq_col[:, 4 + f] = b[E + 128 * f:E + 128 * (f + 1)]
    b_v = b[2 * E:]
    bo_p = (b_v @ np.asarray(W_out, np.float32)
            + np.asarray(b_out, np.float32)).astype(np.float32)
    return {
        "wqkv": np.ascontiguousarray(w.astype(NPBF)),
        "bq": np.ascontiguousarray(bq_col),
        "wout": np.ascontiguousarray(np.asarray(W_out, np.float32).astype(NPBF)),
        "bob": np.ascontiguousarray(np.tile(bo_p[None, :], (128, 1))),
        "ones": np.ones((128, 1), dtype=NPBF),
    }


def _masks_for(h: int) -> np.ndarray:
    """Additive masks [128, 512] bf16: [chunk0 | interior | chunk16].

    Tile element (a, c) of chunk j is key local-row L = 128j + a against
    query local-row r = span_start(j) + 64 + c... computed from first
    principles below.  Valid iff |global q - global k| <= WINDOW and the
    key's global position is inside [0, S)."""
    NEG = np.float32(-1e30)

    def chunk_mask(j, h):
        c0, w = _qspan(j)
        L = 128 * j + np.arange(128)[:, None]      # local key row
        r = 64 + (c0 + np.arange(w))[None, :]      # local query row
        valid = np.abs(r - L) <= WINDOW
        if h == 0:
            valid = valid & (L >= WINDOW)          # global key >= 0
        else:
            valid = valid & (L < ROWS - WINDOW)    # global key < S
        return np.where(valid, np.float32(0.0), NEG)

    m0 = chunk_mask(0, h)                  # [128, 128]
    mi = chunk_mask(8, h)                  # interior pattern, j-independent
    m16 = chunk_mask(NCH - 1, h)           # [128, 128]
    return np.ascontiguousarray(
        np.concatenate([m0, mi, m16], axis=1).astype(NPBF))


def _install_ntff_shim():
    """The agent image's antenv lacks axon_hooks; synthesize it from the
    boot module's ctypes NTFF driver so trace=True can capture HW timing."""
    import types
    if "antenv.axon_hooks" in sys.modules:
        return
    try:
        from trn_agent_boot.trn_boot import _ntff_profile_via_ctypes
        hook = _ntff_profile_via_ctypes("/opt/axon/libaxon_pjrt.so")
    except Exception:
        hook = None
    mod = types.ModuleType("antenv.axon_hooks")
    mod.get_axon_ntff_profile_hook = lambda: hook
    mod.set_axon_ntff_profile_hook = lambda h: None
    sys.modules["antenv.axon_hooks"] = mod
    # avoid S3 artifact upload attempts during local profile processing
    try:
        from concourse import bass_utils as _bu
        _bu.upload_artifacts = lambda tmpdir: tmpdir
    except Exception:
        pass


def kernel(x, W_qkv, b_qkv, W_out, b_out, _trace=False):
    x = np.asarray(x, dtype=np.float32)
    nc = _get_nc()
    shared = _prep_shared(W_qkv, b_qkv, W_out, b_out)
    masks = [_masks_for(0), _masks_for(1)]

    in_maps = []
    for core in range(8):
        b, h = divmod(core, 2)
        lo = h * HALF - WINDOW
        hi = lo + ROWS
        xh = np.zeros((ROWS, E), dtype=np.float32)
        s0, s1 = max(lo, 0), min(hi, S)
        xh[s0 - lo:s1 - lo] = x[b, s0:s1]
        in_maps.append({
            "xT": np.ascontiguousarray(xh.T.astype(NPBF)),
            "masks": masks[h],
            **shared,
        })

    kwargs = {}
    if _trace:
        _install_ntff_shim()
        kwargs = dict(trace=True, trace_cores=[0])
    res = run_bass_kernel_spmd(nc, in_maps, core_ids=list(range(8)), **kwargs)

    out = np.empty((B, S, E), dtype=np.float32)
    for core in range(8):
        b, h = divmod(core, 2)
        out[b, h * HALF:(h + 1) * HALF] = res.results[core]["out"]
    if _trace:
        return out, res
    return out


# revision 7
# speedup vs baseline: 1.4264x; 1.0297x over previous
"""LocalWindowAttention Trainium2 kernel (Bass/Tile), 8-core SPMD — v2.

Problem: x[B=4, S=4096, E=512] -> out[B, S, E]
  qkv = x @ W_qkv + b_qkv ; q,k,v = split(qkv)
  scores = (q @ k.T) / sqrt(E), banded mask |i-j| <= 64, softmax
  out = (attn @ v) @ W_out + b_out

Sharding: 8 cores = (batch b in 0..3) x (seq half h in 0..1). Each core owns
2048 query rows and loads a 64-row halo of x on each side (zero-padded at
sequence boundaries), computing q/k/v locally — no collectives.

v2 design (vs the fp32r v1):
  - All matmul operands are bf16 (PSUM accumulation stays fp32). bf16
    streams 1 col/cycle at ANY moving size (fp32r needs >= 256) and gets
    fast weight loads, halving LDWEIGHTS cost.
  - Scores are computed directly TRANSPOSED, per 128-key chunk j:
    scoresT[k in chunk j, q in its 256-query span] = kT_chunk.T @ qT.
    This kills all 32 PE transposes + their PSUM round trips of v1.
  - Softmax without max-subtraction (scores are O(1)); additive band mask
    on DVE in-place in PSUM, then ACT exp -> bf16 exp tile.
  - Row sums via PE: ones-column matmuls with the exp tiles as stationary
    give rowsum[q, 1] directly in natural orientation; DVE reciprocal.
  - Attention is left UNNORMALIZED through attendedT and the output
    projection; the 1/rowsum scale and the folded output bias
    bo' = b_v @ W_out + b_out (valid because softmax rows sum to 1)
    are applied in ONE fused DVE scalar_tensor_tensor on the final
    PSUM->SBUF copy: out = (po * rd) + bo'.
  - q is only projected for the 2048 owned rows (not the halo).
  - attendedT accumulates the 4 e-chunks into a single PSUM bank using
    per-column-range start=True groups (start clears only the has_written
    bits bank-wide; prior ranges' values are complete and unaffected).
  - PE warm-up: a chain of small matmuls on a zeroed SBUF tile issued
    before any DMA-dependent work, so the HAM clock gate reaches 8/8
    before the real matmuls start, and a dummy Exp to preload the ACT
    table set during the DMA head.
"""

import sys

sys.path.insert(0, "/opt/trn_rl_repo")

import ml_dtypes
import numpy as np

import concourse.bass as bass  # noqa: F401  (registers types)
import concourse.tile as tile
from concourse import bacc, mybir
from concourse.bass_utils import run_bass_kernel_spmd

F32 = mybir.dt.float32
BF16 = mybir.dt.bfloat16
NPBF = ml_dtypes.bfloat16

B, S, E = 4, 4096, 512
WINDOW = 64
HALF = S // 2              # 2048 query rows per core
ROWS = HALF + 2 * WINDOW   # 2176 local rows incl. halo
EC = E // 128              # 4 contraction chunks
NT = HALF // 128           # 16 query subtiles per core
NCH = NT + 1               # 17 key chunks per core
WARMUP_MMS = 64

# exp_all column layout: chunk 0 -> 128 cols, chunks 1..15 -> 256, chunk 16 -> 128
OFF = [0] + [128 + 256 * (j - 1) for j in range(1, NCH)]
WID = [128] + [256] * 15 + [128]
EXPW = OFF[16] + 128       # 4096

_NC_CACHE = {}


def _qspan(j):
    """(first qT column, width) of key chunk j's query span."""
    if j == 0:
        return 0, 128
    if j == NCH - 1:
        return HALF - 128, 128
    return 128 * j - 128, 256


def _build():
    nc = bacc.Bacc("TRN2", target_bir_lowering=False, debug=False, num_devices=8)

    xT_d = nc.dram_tensor("xT", [E, ROWS], BF16, kind="ExternalInput")
    wqkv_d = nc.dram_tensor("wqkv", [E, 3 * E], BF16, kind="ExternalInput")
    bq_d = nc.dram_tensor("bq", [128, 8], F32, kind="ExternalInput")
    wo_d = nc.dram_tensor("wout", [E, E], BF16, kind="ExternalInput")
    bob_d = nc.dram_tensor("bob", [128, E], F32, kind="ExternalInput")
    ones_d = nc.dram_tensor("ones", [128, 1], BF16, kind="ExternalInput")
    mask_d = nc.dram_tensor("masks", [128, 512], BF16, kind="ExternalInput")
    out_d = nc.dram_tensor("out", [HALF, E], F32, kind="ExternalOutput")

    ACT = mybir.ActivationFunctionType
    ALU = mybir.AluOpType

    with tile.TileContext(nc) as tc:
        with (
            tc.tile_pool(name="const", bufs=1) as const,
            tc.tile_pool(name="big", bufs=1) as big,
            tc.tile_pool(name="attn", bufs=2) as attn,
            tc.tile_pool(name="ps", bufs=1, space="PSUM") as psp,
        ):
            # ---- SBUF tiles ----
            wq_sb = [const.tile([128, 3 * E], BF16, name=f"wq{e}", tag=f"wq{e}")
                     for e in range(EC)]
            wo_sb = [const.tile([128, E], BF16, name=f"wo{e}", tag=f"wo{e}")
                     for e in range(EC)]
            bq_sb = const.tile([128, 8], F32, name="bq", tag="bq")
            bob_sb = const.tile([128, E], F32, name="bob", tag="bob")
            ones_sb = const.tile([128, 1], BF16, name="ones1", tag="ones1")
            mask_sb = const.tile([128, 512], BF16, name="msk", tag="msk")
            warm_sb = const.tile([128, 64], BF16, name="warm", tag="warm")
            dummy_sb = const.tile([128, 1], F32, name="dmy", tag="dmy")
            rd_all = const.tile([128, NT], F32, name="rd", tag="rd")

            xT = [big.tile([128, ROWS], BF16, name=f"xT{e}", tag=f"xT{e}")
                  for e in range(EC)]
            qT = [big.tile([128, HALF], BF16, name=f"qT{f}", tag=f"qT{f}")
                  for f in range(EC)]
            kT = [big.tile([128, ROWS], BF16, name=f"kT{f}", tag=f"kT{f}")
                  for f in range(EC)]
            v_sb = [big.tile([128, E], BF16, name=f"v{r}", tag=f"v{r}")
                    for r in range(NCH)]
            exp_all = big.tile([128, EXPW], BF16, name="expa", tag="expa")

            # ---- PE warm-up + ACT table preload (no DMA deps) ----
            nc.vector.memset(warm_sb[:], 0.0)
            nc.scalar.activation(out=dummy_sb[:], in_=warm_sb[:, 0:1],
                                 func=ACT.Exp)
            pw = psp.tile([128, 64], F32, name="pwarm", tag="o", bufs=2)
            for i in range(WARMUP_MMS):
                nc.tensor.matmul(pw[0:64, :], warm_sb[:, 0:64],
                                 warm_sb[:, 0:64], start=True, stop=True)

            # ---- input DMA across 3 queues ----
            # scalar queue: small consts + q-projection weights (needed first)
            nc.scalar.dma_start(out=bq_sb, in_=bq_d[:, :])
            for e in range(EC):
                nc.scalar.dma_start(out=wq_sb[e][:, 0:E],
                                    in_=wqkv_d[128 * e:128 * (e + 1), 0:E])
            # xT: e0/e1 on sync, e2 on gpsimd, e3 on scalar (after wq-q)
            XS = [(0, 512), (512, 512), (1024, 512), (1536, 512), (2048, 128)]
            XQ = {0: nc.sync, 1: nc.sync, 2: nc.gpsimd, 3: nc.scalar}
            for (c0, w) in XS:
                for e in range(EC):
                    XQ[e].dma_start(out=xT[e][:, c0:c0 + w],
                                    in_=xT_d[128 * e:128 * (e + 1), c0:c0 + w])
            nc.scalar.dma_start(out=mask_sb, in_=mask_d[:, :])
            nc.scalar.dma_start(out=ones_sb, in_=ones_d[:, :])
            # gpsimd queue: remaining weights after xT
            for e in range(EC):
                nc.gpsimd.dma_start(out=wq_sb[e][:, E:2 * E],
                                    in_=wqkv_d[128 * e:128 * (e + 1), E:2 * E])
            for e in range(EC):
                nc.gpsimd.dma_start(out=wq_sb[e][:, 2 * E:3 * E],
                                    in_=wqkv_d[128 * e:128 * (e + 1), 2 * E:3 * E])
            for e in range(EC):
                nc.gpsimd.dma_start(out=wo_sb[e],
                                    in_=wo_d[128 * e:128 * (e + 1), :])
            nc.gpsimd.dma_start(out=bob_sb, in_=bob_d[:, :])

            # ---- q projection: qT[f][:, c] = q[row 64+c, 128f+p] ----
            for s in range(4):
                for f in range(EC):
                    ps = psp.tile([128, 512], F32, name=f"pq{s}_{f}", tag="pp", bufs=3)
                    for e in range(EC):
                        nc.tensor.matmul(
                            ps[:],
                            wq_sb[e][:, 128 * f:128 * (f + 1)],
                            xT[e][:, 64 + 512 * s:64 + 512 * (s + 1)],
                            start=(e == 0), stop=(e == EC - 1),
                        )
                    nc.scalar.activation(
                        out=qT[f][:, 512 * s:512 * (s + 1)], in_=ps[:],
                        func=ACT.Identity, bias=bq_sb[:, f:f + 1],
                    )

            # ---- k projection over all 2176 rows ----
            for (c0, w) in XS:
                for f in range(EC):
                    ps = psp.tile([128, 512], F32, name=f"pk{c0}_{f}", tag="pp", bufs=3)
                    for e in range(EC):
                        nc.tensor.matmul(
                            ps[:, :w],
                            wq_sb[e][:, E + 128 * f:E + 128 * (f + 1)],
                            xT[e][:, c0:c0 + w],
                            start=(e == 0), stop=(e == EC - 1),
                        )
                    nc.scalar.activation(
                        out=kT[f][:, c0:c0 + w], in_=ps[:, :w],
                        func=ACT.Identity, bias=bq_sb[:, 4 + f:5 + f],
                    )

            # ---- v projection (natural layout, NO bias — folded into bo') ----
            for r in range(NCH):
                ps = psp.tile([128, 512], F32, name=f"pv{r}", tag="pp", bufs=3)
                for e in range(EC):
                    nc.tensor.matmul(
                        ps[:],
                        xT[e][:, 128 * r:128 * (r + 1)],
                        wq_sb[e][:, 2 * E:3 * E],
                        start=(e == 0), stop=(e == EC - 1),
                    )
                nc.vector.tensor_copy(v_sb[r][:], ps[:])

            # ---- attention ----
            def emit_scores(j):
                c0, w = _qspan(j)
                ps = psp.tile([128, 256], F32, name=f"ps_s{j}", tag="sra",
                              bufs=3)
                for e in range(EC):
                    nc.tensor.matmul(
                        ps[:, :w],
                        kT[e][:, 128 * j:128 * (j + 1)],
                        qT[e][:, c0:c0 + w],
                        start=(e == 0), stop=(e == EC - 1),
                    )
                moff = 0 if j == 0 else (384 if j == NCH - 1 else 128)
                nc.vector.tensor_add(ps[:, :w], ps[:, :w],
                                     mask_sb[:, moff:moff + w])
                nc.scalar.activation(out=exp_all[:, OFF[j]:OFF[j] + w],
                                     in_=ps[:, :w], func=ACT.Exp)

            at_tiles = {}

            def emit_front(t):
                lo = OFF[t] if t == 0 else OFF[t] + 128
                ro = OFF[t + 1]
                # row sums [q, 1] with exp tiles stationary
                pr = psp.tile([128, 1], F32, name=f"ps_r{t}", tag="sra",
                              bufs=3)
                nc.tensor.matmul(pr[:], exp_all[:, lo:lo + 128], ones_sb[:],
                                 start=True, stop=False)
                nc.tensor.matmul(pr[:], exp_all[:, ro:ro + 128], ones_sb[:],
                                 start=False, stop=True)
                nc.vector.reciprocal(rd_all[:, t:t + 1], pr[:])
                # attendedT: 4 e-chunk col ranges in one PSUM bank
                pa = psp.tile([128, 512], F32, name=f"ps_a{t}", tag="sra",
                              bufs=3)
                for c in range(EC):
                    nc.tensor.matmul(
                        pa[:, 128 * c:128 * (c + 1)],
                        v_sb[t][:, 128 * c:128 * (c + 1)],
                        exp_all[:, lo:lo + 128],
                        start=True, stop=False,
                    )
                    nc.tensor.matmul(
                        pa[:, 128 * c:128 * (c + 1)],
                        v_sb[t + 1][:, 128 * c:128 * (c + 1)],
                        exp_all[:, ro:ro + 128],
                        start=False, stop=True,
                    )
                at = attn.tile([128, 512], BF16, name=f"attT{t}", tag="attT",
                               bufs=3)
                nc.scalar.activation(out=at[:], in_=pa[:], func=ACT.Copy)
                at_tiles[t] = at

            def emit_back(t):
                # output projection + fused (x * 1/rowsum) + bo'
                at = at_tiles.pop(t)
                po = psp.tile([128, 512], F32, name=f"ps_o{t}", tag="o",
                              bufs=2)
                for c in range(EC):
                    nc.tensor.matmul(
                        po[:],
                        at[:, 128 * c:128 * (c + 1)],
                        wo_sb[c][:],
                        start=(c == 0), stop=(c == EC - 1),
                    )
                ost = attn.tile([128, 512], F32, name=f"ost{t}", tag="ost")
                nc.vector.scalar_tensor_tensor(
                    ost[:], po[:], rd_all[:, t:t + 1], bob_sb[:],
                    ALU.mult, ALU.add,
                )
                nc.sync.dma_start(out=out_d[128 * t:128 * (t + 1), :],
                                  in_=ost[:])

            for j in range(NCH):
                emit_scores(j)
                if j >= 2:
                    emit_front(j - 2)
                if j >= 3:
                    emit_back(j - 3)
            emit_front(NT - 1)
            emit_back(NT - 2)
            emit_back(NT - 1)

    nc.compile()
    return nc


def _get_nc():
    if "nc" not in _NC_CACHE:
        _NC_CACHE["nc"] = _build()
    return _NC_CACHE["nc"]


def _prep_shared(W_qkv, b_qkv, W_out, b_out):
    scale = 1.0 / np.sqrt(np.float32(E))
    w = np.array(W_qkv, dtype=np.float32, copy=True)
    w[:, :E] *= scale
    b = np.array(b_qkv, dtype=np.float32, copy=True)
    b[:E] *= scale
    bq_col = np.empty((128, 8), dtype=np.float32)
    for f in range(EC):
        bq_col[:, f] = b[128 * f:128 * (f + 1)]
        bq_col[:, 4 + f] = b[E + 128 * f:E + 128 * (f + 1)]
    b_v = b[2 * E:]
    bo_p = (b_v @ np.asarray(W_out, np.float32)
            + np.asarray(b_out, np.float32)).astype(np.float32)
    return {
        "wqkv": np.ascontiguousarray(w.astype(NPBF)),
        "bq": np.ascontiguousarray(bq_col),
        "wout": np.ascontiguousarray(np.asarray(W_out, np.float32).astype(NPBF)),
        "bob": np.ascontiguousarray(np.tile(bo_p[None, :], (128, 1))),
        "ones": np.ones((128, 1), dtype=NPBF),
    }


def _masks_for(h: int) -> np.ndarray:
    """Additive masks [128, 512] bf16: [chunk0 | interior | chunk16].

    Tile element (a, c) of chunk j is key local-row L = 128j + a against
    query local-row r = span_start(j) + 64 + c... computed from first
    principles below.  Valid iff |global q - global k| <= WINDOW and the
    key's global position is inside [0, S)."""
    NEG = np.float32(-1e30)

    def chunk_mask(j, h):
        c0, w = _qspan(j)
        L = 128 * j + np.arange(128)[:, None]      # local key row
        r = 64 + (c0 + np.arange(w))[None, :]      # local query row
        valid = np.abs(r - L) <= WINDOW
        if h == 0:
            valid = valid & (L >= WINDOW)          # global key >= 0
        else:
            valid = valid & (L < ROWS - WINDOW)    # global key < S
        return np.where(valid, np.float32(0.0), NEG)

    m0 = chunk_mask(0, h)                  # [128, 128]
    mi = chunk_mask(8, h)                  # interior pattern, j-independent
    m16 = chunk_mask(NCH - 1, h)           # [128, 128]
    return np.ascontiguousarray(
        np.concatenate([m0, mi, m16], axis=1).astype(NPBF))


def _install_ntff_shim():
    """The agent image's antenv lacks axon_hooks; synthesize it from the
    boot module's ctypes NTFF driver so trace=True can capture HW timing."""
    import types
    if "antenv.axon_hooks" in sys.modules:
        return
    try:
        from trn_agent_boot.trn_boot import _ntff_profile_via_ctypes
        hook = _ntff_profile_via_ctypes("/opt/axon/libaxon_pjrt.so")
    except Exception:
        hook = None
    mod = types.ModuleType("antenv.axon_hooks")
    mod.get_axon_ntff_profile_hook = lambda: hook
    mod.set_axon_ntff_profile_hook = lambda h: None
    sys.modules["antenv.axon_hooks"] = mod
    # avoid S3 artifact upload attempts during local profile processing
    try:
        from concourse import bass_utils as _bu
        _bu.upload_artifacts = lambda tmpdir: tmpdir
    except Exception:
        pass


def kernel(x, W_qkv, b_qkv, W_out, b_out, _trace=False):
    x = np.asarray(x, dtype=np.float32)
    nc = _get_nc()
    shared = _prep_shared(W_qkv, b_qkv, W_out, b_out)
    masks = [_masks_for(0), _masks_for(1)]

    in_maps = []
    for core in range(8):
        b, h = divmod(core, 2)
        lo = h * HALF - WINDOW
        hi = lo + ROWS
        xh = np.zeros((ROWS, E), dtype=np.float32)
        s0, s1 = max(lo, 0), min(hi, S)
        xh[s0 - lo:s1 - lo] = x[b, s0:s1]
        in_maps.append({
            "xT": np.ascontiguousarray(xh.T.astype(NPBF)),
            "masks": masks[h],
            **shared,
        })

    kwargs = {}
    if _trace:
        _install_ntff_shim()
        kwargs = dict(trace=True, trace_cores=[0])
    res = run_bass_kernel_spmd(nc, in_maps, core_ids=list(range(8)), **kwargs)

    out = np.empty((B, S, E), dtype=np.float32)
    for core in range(8):
        b, h = divmod(core, 2)
        out[b, h * HALF:(h + 1) * HALF] = res.results[core]["out"]
    if _trace:
        return out, res
    return out


# revision 11
# speedup vs baseline: 1.4273x; 1.0006x over previous
"""LocalWindowAttention Trainium2 kernel (Bass/Tile), 8-core SPMD — v2.

Problem: x[B=4, S=4096, E=512] -> out[B, S, E]
  qkv = x @ W_qkv + b_qkv ; q,k,v = split(qkv)
  scores = (q @ k.T) / sqrt(E), banded mask |i-j| <= 64, softmax
  out = (attn @ v) @ W_out + b_out

Sharding: 8 cores = (batch b in 0..3) x (seq half h in 0..1). Each core owns
2048 query rows and loads a 64-row halo of x on each side (zero-padded at
sequence boundaries), computing q/k/v locally — no collectives.

v2 design (vs the fp32r v1):
  - All matmul operands are bf16 (PSUM accumulation stays fp32). bf16
    streams 1 col/cycle at ANY moving size (fp32r needs >= 256) and gets
    fast weight loads, halving LDWEIGHTS cost.
  - Scores are computed directly TRANSPOSED, per 128-key chunk j:
    scoresT[k in chunk j, q in its 256-query span] = kT_chunk.T @ qT.
    This kills all 32 PE transposes + their PSUM round trips of v1.
  - Softmax without max-subtraction (scores are O(1)); additive band mask
    on DVE in-place in PSUM, then ACT exp -> bf16 exp tile.
  - Row sums via PE: ones-column matmuls with the exp tiles as stationary
    give rowsum[q, 1] directly in natural orientation; DVE reciprocal.
  - Attention is left UNNORMALIZED through attendedT and the output
    projection; the 1/rowsum scale and the folded output bias
    bo' = b_v @ W_out + b_out (valid because softmax rows sum to 1)
    are applied in ONE fused DVE scalar_tensor_tensor on the final
    PSUM->SBUF copy: out = (po * rd) + bo'.
  - q is only projected for the 2048 owned rows (not the halo).
  - attendedT accumulates the 4 e-chunks into a single PSUM bank using
    per-column-range start=True groups (start clears only the has_written
    bits bank-wide; prior ranges' values are complete and unaffected).
  - PE warm-up: a chain of small matmuls on a zeroed SBUF tile issued
    before any DMA-dependent work, so the HAM clock gate reaches 8/8
    before the real matmuls start, and a dummy Exp to preload the ACT
    table set during the DMA head.
"""

import sys

sys.path.insert(0, "/opt/trn_rl_repo")

import ml_dtypes
import numpy as np

import concourse.bass as bass  # noqa: F401  (registers types)
import concourse.tile as tile
from concourse import bacc, mybir
from concourse.bass_utils import run_bass_kernel_spmd

F32 = mybir.dt.float32
BF16 = mybir.dt.bfloat16
NPBF = ml_dtypes.bfloat16

B, S, E = 4, 4096, 512
WINDOW = 64
HALF = S // 2              # 2048 query rows per core
ROWS = HALF + 2 * WINDOW   # 2176 local rows incl. halo
EC = E // 128              # 4 contraction chunks
NT = HALF // 128           # 16 query subtiles per core
NCH = NT + 1               # 17 key chunks per core
WARMUP_MMS = 56

# exp_all column layout: chunk 0 -> 128 cols, chunks 1..15 -> 256, chunk 16 -> 128
OFF = [0] + [128 + 256 * (j - 1) for j in range(1, NCH)]
WID = [128] + [256] * 15 + [128]
EXPW = OFF[16] + 128       # 4096

_NC_CACHE = {}


def _qspan(j):
    """(first qT column, width) of key chunk j's query span."""
    if j == 0:
        return 0, 128
    if j == NCH - 1:
        return HALF - 128, 128
    return 128 * j - 128, 256


def _build():
    nc = bacc.Bacc("TRN2", target_bir_lowering=False, debug=False, num_devices=8)

    xT_d = nc.dram_tensor("xT", [E, ROWS], BF16, kind="ExternalInput")
    wqkv_d = nc.dram_tensor("wqkv", [E, 3 * E], BF16, kind="ExternalInput")
    bq_d = nc.dram_tensor("bq", [128, 8], F32, kind="ExternalInput")
    wo_d = nc.dram_tensor("wout", [E, E], BF16, kind="ExternalInput")
    bob_d = nc.dram_tensor("bob", [128, E], F32, kind="ExternalInput")
    ones_d = nc.dram_tensor("ones", [128, 1], BF16, kind="ExternalInput")
    mask_d = nc.dram_tensor("masks", [128, 512], BF16, kind="ExternalInput")
    out_d = nc.dram_tensor("out", [HALF, E], F32, kind="ExternalOutput")

    ACT = mybir.ActivationFunctionType
    ALU = mybir.AluOpType

    with tile.TileContext(nc) as tc:
        with (
            tc.tile_pool(name="const", bufs=1) as const,
            tc.tile_pool(name="big", bufs=1) as big,
            tc.tile_pool(name="attn", bufs=2) as attn,
            tc.tile_pool(name="ps", bufs=1, space="PSUM") as psp,
        ):
            # ---- SBUF tiles ----
            wq_sb = [const.tile([128, 3 * E], BF16, name=f"wq{e}", tag=f"wq{e}")
                     for e in range(EC)]
            wo_sb = [const.tile([128, E], BF16, name=f"wo{e}", tag=f"wo{e}")
                     for e in range(EC)]
            bq_sb = const.tile([128, 8], F32, name="bq", tag="bq")
            bob_sb = const.tile([128, E], F32, name="bob", tag="bob")
            ones_sb = const.tile([128, 1], BF16, name="ones1", tag="ones1")
            mask_sb = const.tile([128, 512], BF16, name="msk", tag="msk")
            warm_sb = const.tile([128, 64], BF16, name="warm", tag="warm")
            dummy_sb = const.tile([128, 1], F32, name="dmy", tag="dmy")
            rd_all = const.tile([128, NT], F32, name="rd", tag="rd")

            xT = [big.tile([128, ROWS], BF16, name=f"xT{e}", tag=f"xT{e}")
                  for e in range(EC)]
            qT = [big.tile([128, HALF], BF16, name=f"qT{f}", tag=f"qT{f}")
                  for f in range(EC)]
            kT = [big.tile([128, ROWS], BF16, name=f"kT{f}", tag=f"kT{f}")
                  for f in range(EC)]
            v_sb = [big.tile([128, E], BF16, name=f"v{r}", tag=f"v{r}")
                    for r in range(NCH)]
            exp_all = big.tile([128, EXPW], BF16, name="expa", tag="expa")

            # ---- PE warm-up + ACT table preload (no DMA deps) ----
            nc.vector.memset(warm_sb[:], 0.0)
            nc.scalar.activation(out=dummy_sb[:], in_=warm_sb[:, 0:1],
                                 func=ACT.Exp)
            pw = psp.tile([128, 64], F32, name="pwarm", tag="o", bufs=2)
            for i in range(WARMUP_MMS):
                nc.tensor.matmul(pw[0:64, :], warm_sb[:, 0:64],
                                 warm_sb[:, 0:64], start=True, stop=True)

            # ---- input DMA round-robined over the 3 DMA-capable queues ----
            # 64-aligned xT slices so each q-proj group depends on ONE slice
            XS = [(0, 576), (576, 512), (1088, 512), (1600, 512), (2112, 64)]
            nc.scalar.dma_start(out=bq_sb, in_=bq_d[:, :])
            nc.scalar.dma_start(out=mask_sb, in_=mask_d[:, :])
            nc.scalar.dma_start(out=ones_sb, in_=ones_d[:, :])
            queues = [nc.sync, nc.gpsimd, nc.scalar]
            big = []
            for e in range(EC):
                big.append((wq_sb[e][:, 0:E],
                            wqkv_d[128 * e:128 * (e + 1), 0:E]))
            for (c0, w) in XS:
                for e in range(EC):
                    big.append((xT[e][:, c0:c0 + w],
                                xT_d[128 * e:128 * (e + 1), c0:c0 + w]))
            for e in range(EC):
                big.append((wq_sb[e][:, E:2 * E],
                            wqkv_d[128 * e:128 * (e + 1), E:2 * E]))
            for e in range(EC):
                big.append((wq_sb[e][:, 2 * E:3 * E],
                            wqkv_d[128 * e:128 * (e + 1), 2 * E:3 * E]))
            for e in range(EC):
                big.append((wo_sb[e][:], wo_d[128 * e:128 * (e + 1), :]))
            big.append((bob_sb[:], bob_d[:, :]))
            for i, (dst, src) in enumerate(big):
                queues[i % 3].dma_start(out=dst, in_=src)

            # ---- q projection: qT[f][:, c] = q[row 64+c, 128f+p] ----
            for s in range(4):
                for f in range(EC):
                    ps = psp.tile([128, 512], F32, name=f"pq{s}_{f}", tag="pp", bufs=3)
                    for e in range(EC):
                        nc.tensor.matmul(
                            ps[:],
                            wq_sb[e][:, 128 * f:128 * (f + 1)],
                            xT[e][:, 64 + 512 * s:64 + 512 * (s + 1)],
                            start=(e == 0), stop=(e == EC - 1),
                        )
                    nc.scalar.activation(
                        out=qT[f][:, 512 * s:512 * (s + 1)], in_=ps[:],
                        func=ACT.Identity, bias=bq_sb[:, f:f + 1],
                    )

            # ---- k projection over all 2176 rows ----
            KS = [(0, 512), (512, 512), (1024, 512), (1536, 512), (2048, 128)]
            for (c0, w) in KS:
                for f in range(EC):
                    ps = psp.tile([128, 512], F32, name=f"pk{c0}_{f}", tag="pp", bufs=3)
                    for e in range(EC):
                        nc.tensor.matmul(
                            ps[:, :w],
                            wq_sb[e][:, E + 128 * f:E + 128 * (f + 1)],
                            xT[e][:, c0:c0 + w],
                            start=(e == 0), stop=(e == EC - 1),
                        )
                    nc.scalar.activation(
                        out=kT[f][:, c0:c0 + w], in_=ps[:, :w],
                        func=ACT.Identity, bias=bq_sb[:, 4 + f:5 + f],
                    )

            # ---- v projection (natural layout, NO bias — folded into bo') ----
            for r in range(NCH):
                ps = psp.tile([128, 512], F32, name=f"pv{r}", tag="pp", bufs=3)
                for e in range(EC):
                    nc.tensor.matmul(
                        ps[:],
                        xT[e][:, 128 * r:128 * (r + 1)],
                        wq_sb[e][:, 2 * E:3 * E],
                        start=(e == 0), stop=(e == EC - 1),
                    )
                nc.vector.tensor_copy(v_sb[r][:], ps[:])

            # ---- attention ----
            def emit_scores(j):
                c0, w = _qspan(j)
                ps = psp.tile([128, 256], F32, name=f"ps_s{j}", tag="sra",
                              bufs=3)
                for e in range(EC):
                    nc.tensor.matmul(
                        ps[:, :w],
                        kT[e][:, 128 * j:128 * (j + 1)],
                        qT[e][:, c0:c0 + w],
                        start=(e == 0), stop=(e == EC - 1),
                    )
                moff = 0 if j == 0 else (384 if j == NCH - 1 else 128)
                nc.vector.tensor_add(ps[:, :w], ps[:, :w],
                                     mask_sb[:, moff:moff + w])
                nc.scalar.activation(out=exp_all[:, OFF[j]:OFF[j] + w],
                                     in_=ps[:, :w], func=ACT.Exp)

            at_tiles = {}

            def emit_front(t):
                lo = OFF[t] if t == 0 else OFF[t] + 128
                ro = OFF[t + 1]
                # row sums [q, 1] with exp tiles stationary
                pr = psp.tile([128, 1], F32, name=f"ps_r{t}", tag="sra",
                              bufs=3)
                nc.tensor.matmul(pr[:], exp_all[:, lo:lo + 128], ones_sb[:],
                                 start=True, stop=False)
                nc.tensor.matmul(pr[:], exp_all[:, ro:ro + 128], ones_sb[:],
                                 start=False, stop=True)
                nc.vector.reciprocal(rd_all[:, t:t + 1], pr[:])
                # attendedT: 4 e-chunk col ranges in one PSUM bank
                pa = psp.tile([128, 512], F32, name=f"ps_a{t}", tag="sra",
                              bufs=3)
                for c in range(EC):
                    nc.tensor.matmul(
                        pa[:, 128 * c:128 * (c + 1)],
                        v_sb[t][:, 128 * c:128 * (c + 1)],
                        exp_all[:, lo:lo + 128],
                        start=True, stop=False,
                    )
                    nc.tensor.matmul(
                        pa[:, 128 * c:128 * (c + 1)],
                        v_sb[t + 1][:, 128 * c:128 * (c + 1)],
                        exp_all[:, ro:ro + 128],
                        start=False, stop=True,
                    )
                at = attn.tile([128, 512], BF16, name=f"attT{t}", tag="attT",
                               bufs=3)
                nc.scalar.activation(out=at[:], in_=pa[:], func=ACT.Copy)
                at_tiles[t] = at

            def emit_back(t):
                # output projection + fused (x * 1/rowsum) + bo'
                at = at_tiles.pop(t)
                po = psp.tile([128, 512], F32, name=f"ps_o{t}", tag="o",
                              bufs=2)
                for c in range(EC):
                    nc.tensor.matmul(
                        po[:],
                        at[:, 128 * c:128 * (c + 1)],
                        wo_sb[c][:],
                        start=(c == 0), stop=(c == EC - 1),
                    )
                ost = attn.tile([128, 512], F32, name=f"ost{t}", tag="ost")
                nc.vector.scalar_tensor_tensor(
                    ost[:], po[:], rd_all[:, t:t + 1], bob_sb[:],
                    ALU.mult, ALU.add,
                )
                nc.sync.dma_start(out=out_d[128 * t:128 * (t + 1), :],
                                  in_=ost[:])

            for j in range(NCH):
                emit_scores(j)
                if j >= 2:
                    emit_front(j - 2)
                if j >= 3:
                    emit_back(j - 3)
            emit_front(NT - 1)
            emit_back(NT - 2)
            emit_back(NT - 1)

    nc.compile()
    return nc


def _get_nc():
    if "nc" not in _NC_CACHE:
        _NC_CACHE["nc"] = _build()
    return _NC_CACHE["nc"]


def _prep_shared(W_qkv, b_qkv, W_out, b_out):
    scale = 1.0 / np.sqrt(np.float32(E))
    w = np.array(W_qkv, dtype=np.float32, copy=True)
    w[:, :E] *= scale
    b = np.array(b_qkv, dtype=np.float32, copy=True)
    b[:E] *= scale
    bq_col = np.empty((128, 8), dtype=np.float32)
    for f in range(EC):
        bq_col[:, f] = b[128 * f:128 * (f + 1)]
        bq_col[:, 4 + f] = b[E + 128 * f:E + 128 * (f + 1)]
    b_v = b[2 * E:]
    bo_p = (b_v @ np.asarray(W_out, np.float32)
            + np.asarray(b_out, np.float32)).astype(np.float32)
    return {
        "wqkv": np.ascontiguousarray(w.astype(NPBF)),
        "bq": np.ascontiguousarray(bq_col),
        "wout": np.ascontiguousarray(np.asarray(W_out, np.float32).astype(NPBF)),
        "bob": np.ascontiguousarray(np.tile(bo_p[None, :], (128, 1))),
        "ones": np.ones((128, 1), dtype=NPBF),
    }


def _masks_for(h: int) -> np.ndarray:
    """Additive masks [128, 512] bf16: [chunk0 | interior | chunk16].

    Tile element (a, c) of chunk j is key local-row L = 128j + a against
    query local-row r = span_start(j) + 64 + c... computed from first
    principles below.  Valid iff |global q - global k| <= WINDOW and the
    key's global position is inside [0, S)."""
    NEG = np.float32(-1e30)

    def chunk_mask(j, h):
        c0, w = _qspan(j)
        L = 128 * j + np.arange(128)[:, None]      # local key row
        r = 64 + (c0 + np.arange(w))[None, :]      # local query row
        valid = np.abs(r - L) <= WINDOW
        if h == 0:
            valid = valid & (L >= WINDOW)          # global key >= 0
        else:
            valid = valid & (L < ROWS - WINDOW)    # global key < S
        return np.where(valid, np.float32(0.0), NEG)

    m0 = chunk_mask(0, h)                  # [128, 128]
    mi = chunk_mask(8, h)                  # interior pattern, j-independent
    m16 = chunk_mask(NCH - 1, h)           # [128, 128]
    return np.ascontiguousarray(
        np.concatenate([m0, mi, m16], axis=1).astype(NPBF))


def _install_ntff_shim():
    """The agent image's antenv lacks axon_hooks; synthesize it from the
    boot module's ctypes NTFF driver so trace=True can capture HW timing."""
    import types
    if "antenv.axon_hooks" in sys.modules:
        return
    try:
        from trn_agent_boot.trn_boot import _ntff_profile_via_ctypes
        hook = _ntff_profile_via_ctypes("/opt/axon/libaxon_pjrt.so")
    except Exception:
        hook = None
    mod = types.ModuleType("antenv.axon_hooks")
    mod.get_axon_ntff_profile_hook = lambda: hook
    mod.set_axon_ntff_profile_hook = lambda h: None
    sys.modules["antenv.axon_hooks"] = mod
    # avoid S3 artifact upload attempts during local profile processing
    try:
        from concourse import bass_utils as _bu
        _bu.upload_artifacts = lambda tmpdir: tmpdir
    except Exception:
        pass


def kernel(x, W_qkv, b_qkv, W_out, b_out, _trace=False):
    x = np.asarray(x, dtype=np.float32)
    nc = _get_nc()
    shared = _prep_shared(W_qkv, b_qkv, W_out, b_out)
    masks = [_masks_for(0), _masks_for(1)]

    in_maps = []
    for core in range(8):
        b, h = divmod(core, 2)
        lo = h * HALF - WINDOW
        hi = lo + ROWS
        xh = np.zeros((ROWS, E), dtype=np.float32)
        s0, s1 = max(lo, 0), min(hi, S)
        xh[s0 - lo:s1 - lo] = x[b, s0:s1]
        in_maps.append({
            "xT": np.ascontiguousarray(xh.T.astype(NPBF)),
            "masks": masks[h],
            **shared,
        })

    kwargs = {}
    if _trace:
        _install_ntff_shim()
        kwargs = dict(trace=True, trace_cores=[0])
    res = run_bass_kernel_spmd(nc, in_maps, core_ids=list(range(8)), **kwargs)

    out = np.empty((B, S, E), dtype=np.float32)
    for core in range(8):
        b, h = divmod(core, 2)
        out[b, h * HALF:(h + 1) * HALF] = res.results[core]["out"]
    if _trace:
        return out, res
    return out
